# revision 40
# baseline (speedup 1.0000x reference)
"""Segment-max pooling (wordpiece->word) Bass kernel for TRN2, 8 cores.

Strategy: pure data parallel, 2 examples per core. Per example:
  - nonempty spans are split into pow2 length classes R in {1,2,4,8}
    (spans longer than RMAX=8 are chained through extra rows and
    max-combined on the host),
  - each class is sorted by length (desc) and packed into groups of
    <=128 lanes (one span per SBUF partition),
  - per group, `rnds` indirect DMA gathers pull the span tokens from
    the context table in HBM into disjoint slices of a [128,rnds,1024]
    SBUF tile (all gathers of a group run concurrently),
  - one strided in-place vector reduce_max folds the rounds axis into
    slice 0,
  - one plain DMA stores slice 0 to a per-group DRAM output tensor.
The host permutes group lanes back to span slots and assembles the
[B,S,D] zero-padded result.

Sync-wait budget: the walrus codegen used by the bass2jax/axon path
allows a single attached sync wait per instruction; _split_waits hoists
any extra Tile-generated waits into standalone EventSemaphore
instructions on the same engine queue. Per-group output tensors avoid
WAW serialization between stores.
"""

import sys

if "/opt/trn_rl_repo" not in sys.path:
    sys.path.insert(0, "/opt/trn_rl_repo")

import numpy as np

B, S, D, N = 16, 4096, 1024, 1024
NCORES = 8
EPC = B // NCORES  # examples per core
RMAX = 8
PAD_GIDX = 100000  # > EPC*S-1, within int32 after *D
CLASSES = (8, 4, 2)

_CACHE = {}
LAST_RESULTS = None


def _plan(spans):
    spans = np.asarray(spans).astype(np.int64)
    per_ex = []
    for b in range(B):
        st = spans[b, :, 0]
        ln = spans[b, :, 1] - st
        subs = {R: [] for R in CLASSES}
        fix = []  # (span_i, pooled_rows, direct_tokens) host combine entries
        chain = 0
        for i in np.nonzero(ln > 0)[0]:
            s = int(st[i])
            l = int(ln[i])
            if l == 1:
                # singleton span: the row is a verbatim context row; the
                # host fills it during assembly -- no device traffic
                fix.append((int(i), [], [s]))
            elif l <= RMAX:
                R = min(c for c in CLASSES if c >= l)
                subs[R].append((s, l, int(i)))
            else:
                rows = []
                toks = []
                for o in range(0, l, RMAX):
                    ls = min(RMAX, l - o)
                    if ls == 1:
                        toks.append(s + o)
                        continue
                    R = min(c for c in CLASSES if c >= ls)
                    row = N + chain
                    chain += 1
                    subs[R].append((s + o, ls, row))
                    rows.append(row)
                fix.append((int(i), rows, toks))
        for R in CLASSES:
            subs[R].sort(key=lambda t: -t[1])
        per_ex.append((subs, fix))

    calls = []  # static: (R, g, npg, rounds)
    for R in CLASSES:
        gmax = max(-(-len(p[0][R]) // 128) for p in per_ex)
        for g in range(gmax):
            npg = max(min(max(len(p[0][R]) - g * 128, 0), 128) for p in per_ex)
            rnds = max(
                (p[0][R][g * 128][1] if len(p[0][R]) > g * 128 else 0)
                for p in per_ex
            )
            if npg and rnds:
                calls.append((R, g, npg, rnds))
    gcols = sum(c[3] for c in calls)
    ngroups = len(calls)

    # pad lanes get an out-of-bounds index; the gather's bounds_check
    # silently skips them (no HBM traffic, lane ignored downstream)
    gidx = np.full((NCORES, 128, EPC * gcols), PAD_GIDX, np.int32)
    # host-side lane -> output row map per (example, group); -1 = pad
    lanemap = np.full((B, ngroups, 128), -1, np.int64)
    for b in range(B):
        c, e = divmod(b, EPC)
        subs = per_ex[b][0]
        col = e * gcols
        for gi, (R, g, npg, rnds) in enumerate(calls):
            lanes = subs[R][g * 128 : g * 128 + 128]
            for p, (s, l, row) in enumerate(lanes):
                lanemap[b, gi, p] = row
            for r in range(rnds):
                for p, (s, l, row) in enumerate(lanes):
                    gidx[c, p, col] = e * S + s + min(r, l - 1)
                col += 1
    fixups = [p[1] for p in per_ex]
    nchain = max((max((r for _, rows, _t in f for r in rows), default=N - 1) for f in fixups), default=N - 1) - N + 1
    sig = tuple(calls)
    return sig, calls, gcols, ngroups, gidx, lanemap, fixups, nchain


def _split_waits(nc):
    """Give every instruction at most one attached sync wait.

    The walrus codegen used by the bass2jax/axon path accepts a single
    sync-wait command per instruction, but Tile's add_semaphores may
    attach several (multiple DMA completion lanes, cross-engine deps).
    Semantics-preserving fix: keep one wait attached and hoist the rest
    into standalone InstEventSemaphore instructions inserted directly
    before the instruction on the same engine queue -- the sequencer
    executes them in order, so the wait set is unchanged.
    """
    from concourse import mybir

    # a sem id no instruction in the final program references (Tile
    # released its sems post-schedule, so the allocator would hand back
    # a live DMA-lane id)
    used = set()
    for bb in nc.main_func.blocks:
        for ins in bb.instructions:
            si = ins.sync_info
            if si is not None:
                for w in si.on_wait:
                    used.add(w.id)
                for u in si.on_update:
                    used.add(u.id)
    ws_id = max(used) + 1 if used else 0
    for bb in nc.main_func.blocks:
        insts = bb.instructions
        targets = []
        for pos, ins in enumerate(insts):
            si = ins.sync_info
            if si is not None and len(si.on_wait) > 1:
                targets.append((pos, ins))
        for pos, ins in reversed(targets):
            si = ins.sync_info
            waits = list(si.on_wait)
            keep = waits[-1]
            extra = waits[:-1]
            while len(si.on_wait) > 0:
                si.on_wait.pop()
            si.on_wait.append(keep)
            SyncInfo = type(si)
            SyncUpdate = type(si.on_update[0]) if si.on_update else None
            for k, w in enumerate(extra):
                ev = mybir.InstEventSemaphore(name=f"WS{k}-{ins.name}")
                ev.engine = ins.engine
                # the sim requires every executable instruction to have an
                # on_update; inc a dedicated sem nothing waits on
                upd = (
                    [
                        SyncUpdate(
                            sync_type="semaphore",
                            id=ws_id,
                            ant_name="ws_split",
                            update_mode="sem-inc",
                            update_value=1,
                        )
                    ]
                    if SyncUpdate is not None
                    else []
                )
                ev.sync_info = SyncInfo(on_wait=[w], on_update=upd)
                insts.insert(pos, ev)
                nc.inst_map[ev.name] = ev
    return nc


def _build(calls, gcols, ngroups):
    from concourse import bass, mybir, tile

    nc = bass.Bass()
    f32 = mybir.dt.float32
    i32 = mybir.dt.int32
    ctx_t = nc.declare_dram_parameter("ctx", [EPC * S, D], f32, isOutput=False)
    gidx_t = nc.declare_dram_parameter("gidx", [128, EPC * gcols], i32, isOutput=False)
    out_t = [
        nc.declare_dram_parameter(f"out{e}_{gi}", [128, D], f32, isOutput=True)
        for e in range(EPC)
        for gi in range(ngroups)
    ]
    colbase = []
    acc = 0
    for _R, _g, _npg, rnds in calls:
        colbase.append(acc)
        acc += rnds
    with tile.TileContext(nc) as tc:
        with (
            tc.tile_pool(name="sbuf", bufs=1) as pool,
            tc.tile_pool(name="scratch", bufs=3) as spool,
        ):
            nc.gpsimd.preamble()  # register init for bounds_check scalars
            breg = nc.gpsimd.to_reg(EPC * S - 1)  # shared bounds register
            gt = pool.tile([128, EPC * gcols], i32, tag="gidx")
            # per-example idx loads so example 0's gathers start sooner
            for e in range(EPC):
                nc.sync.dma_start(
                    out=gt[:, e * gcols : (e + 1) * gcols],
                    in_=gidx_t[:, e * gcols : (e + 1) * gcols],
                )
            # interleave the two examples' groups for tighter packing
            for gi, (R, g, npg, rnds) in enumerate(calls):
                for e in range(EPC):
                    col = e * gcols + colbase[gi]
                    # all rounds gather concurrently into disjoint slices
                    # of one wide tile; one strided in-place reduce folds
                    # them into slice 0
                    wide = spool.tile([128, rnds, D], f32, tag=f"w{R}")
                    for r in range(rnds):
                        nc.gpsimd.indirect_dma_start(
                            out=wide[0:npg, r, :],
                            out_offset=None,
                            in_=ctx_t[:],
                            in_offset=bass.IndirectOffsetOnAxis(
                                ap=gt[0:npg, col + r : col + r + 1], axis=0
                            ),
                            bounds_check=breg,
                            oob_is_err=False,
                        )
                    if rnds > 1:
                        nc.vector.reduce_max(
                            out=wide[0:npg, 0, :],
                            in_=wide[0:npg].transpose([0, 2, 1]),
                            axis=mybir.AxisListType.X,
                        )
                    # HWDGE store: keeps the big writes off the SWDGE
                    # track so they overlap the gathers
                    nc.sync.dma_start(
                        out=out_t[e * ngroups + gi][0:npg, :],
                        in_=wide[0:npg, 0, :],
                    )
    return _split_waits(nc)


def kernel(context, spans, trace=False):
    global LAST_RESULTS
    context = np.ascontiguousarray(np.asarray(context, dtype=np.float32))
    spans_np = np.asarray(spans)
    sig, calls, gcols, ngroups, gidx, lanemap, fixups, nchain = _plan(spans_np)
    if ngroups == 0:
        # every nonempty span is a singleton (or there are none):
        # assembly is purely host-side
        out = np.zeros((B, S, D), np.float32)
        for b in range(B):
            for i, rows, toks in fixups[b]:
                out[b, i] = context[b, toks].max(axis=0)
        return out
    if sig not in _CACHE:
        _CACHE[sig] = _build(calls, gcols, ngroups)
    nc = _CACHE[sig]

    from concourse.bass_utils import run_bass_kernel_spmd

    in_maps = [
        {
            "ctx": context[c * EPC : (c + 1) * EPC].reshape(EPC * S, D),
            "gidx": gidx[c],
        }
        for c in range(NCORES)
    ]
    LAST_RESULTS = run_bass_kernel_spmd(
        nc, in_maps, list(range(NCORES)), trace=trace
    )
    res = LAST_RESULTS.results

    out = np.zeros((B, S, D), np.float32)
    pooled = np.zeros((N + nchain, D), np.float32)
    for b in range(B):
        c, e = divmod(b, EPC)
        pooled[:] = 0.0
        for gi in range(ngroups):
            rows = lanemap[b, gi]
            valid = rows >= 0
            if valid.any():
                pooled[rows[valid]] = res[c][f"out{e}_{gi}"][: len(valid)][valid]
        out[b, :N] = pooled[:N]
        for i, rows, toks in fixups[b]:
            cands = []
            if rows:
                cands.append(pooled[rows].max(axis=0))
            if toks:
                cands.append(context[b, toks].max(axis=0))
            out[b, i] = cands[0] if len(cands) == 1 else np.maximum(cands[0], cands[1])
    return out
